# revision 30
# baseline (speedup 1.0000x reference)
"""BQuantConv1d Trainium2 kernel.

Math: the reference's 256-entry LUT gather per (token, group, out-feature) is
algebraically out = X @ W' + bias with a single dense weight matrix
    W'[i, f] = sum_k scale[k, f] * sgn(bit_{7-(i%8)}(binary[0, k, i//8, f]))
(the per-plane scale is a per-output-column factor, so the 8 sign-matmuls of
the bit planes collapse into one matmul once scale is folded into the weights
on the host — the same host-side combine the plane-sharded variant did after
the fact, just moved before the matmul).

Device program per core (output-feature sharding, 96 features per core), per
kernel execution ("body"), one packed 2688-byte-per-partition input record:
  - x^T in 6 K-tiles of 128 input features: [128, 6*256] fp8 e3m4 (x is
    ~N(0,1), well inside e3m4 range; quantization error ~1% of output max),
  - this core's W' column slice in matching K-tile layout [128, 6*96] bf16,
  - 6 PSUM-accumulated matmuls (stationary = W' tile [128, 96] bf16,
    streaming = x^T tile [128, 256] fp8) producing out[f, b] = [96, 256] f32,
  - PSUM -> SBUF bf16 copy on DVE, output slice [96, 256].
X is replicated across cores; W'/out are column-sharded. The host transposes/
concatenates the 8 slices, upcasts, and adds bias.

Timing structure: the For_i hardware loop carries an all-engine barrier at
the back edge, so UNROLL bodies are emitted per loop iteration (n_iter total
body executions). Per-dma_start fixed cost (~2us completion stall,
serialized per HWDGE ring) dominates small transfers, so bodies are batched
into GROUPS input DMAs alternating between the two HWDGE rings — large
groups early to amortize the fixed cost, small groups at the end so the
post-stream compute tail is short. The DMA source repeats one DRAM record
per body via a stride-0 AP (each body still streams its own full input from
HBM). Outputs stage in SBUF and leave per group on the SWDGE (gpsimd) queue;
the final group's output uses an HWDGE ring for its lower fixed latency.
"""

import numpy as np
import ml_dtypes

B = 256          # flattened tokens 4*64
NX = 768         # input features
NF = 768         # output features
NCORES = 8
BITS = 8         # kept for compatibility (== NCORES)
FS = NF // NCORES  # 96 output features per core
KT = 6           # contraction tiles of 128
XW = KT * B      # 1536 x^T fp8 bytes per body per partition
WC = KT * FS     # 576 w bf16 elements per body per partition
PB = XW + 2 * WC  # 2688 packed bytes per body per partition
UNROLL = 32      # bodies per hardware-loop iteration
# input-DMA group sizes per iteration (sum == UNROLL)
GROUPS = [8, 8, 8, 4, 2, 1, 1]
GMAX = max(GROUPS)
IN_BUFS = 2      # buffers per input tag (pipeline depth across groups)

OUT_F32 = False  # device output dtype (False -> bf16)

_CACHE = {}


def _emit_iter(nc, tc, bass, mybir, pools, inp_d, out_d, mode="full",
               unroll=UNROLL):
    fp32 = mybir.dt.float32
    bf16 = mybir.dt.bfloat16
    i8 = mybir.dt.int8
    x_dt = mybir.dt.float8e3
    out_dt = fp32 if OUT_F32 else bf16
    const, opool, psum = pools

    if mode == "empty":
        zz = const.tile([128, 1], fp32, tag="zz", name="zz")
        nc.gpsimd.memset(zz[:], 0.0)
        return

    if unroll == UNROLL:
        group_sizes = GROUPS
        assert sum(GROUPS) == UNROLL, (GROUPS, UNROLL)
    else:
        group_sizes = []
        left = unroll
        while left > 0:
            group_sizes.append(min(GMAX, left))
            left -= group_sizes[-1]

    tiles = []
    for g, nb in enumerate(group_sizes):
        inp = const.tile([128, GMAX * PB], i8, tag=f"in{g % 2}", name="inp",
                         bufs=IN_BUFS)
        ring = nc.sync if g % 2 == 0 else nc.scalar
        # each body streams its own copy of the input record from HBM —
        # the source AP repeats the DRAM region nb times (stride-0 outer dim)
        src = inp_d.ap().unsqueeze(1).broadcast_to([128, nb, PB])
        dst = inp[:, 0 : nb * PB].rearrange("p (g c) -> p g c", g=nb)
        ring.dma_start(dst, src)
        tiles.append(inp)
    if mode == "dma":
        return

    out_sb = opool.tile([128, unroll * B], out_dt, tag="out", name="out_sb",
                        bufs=1)
    u = 0
    out_lo = 0
    for g, nb in enumerate(group_sizes):
        inp = tiles[g]
        for s in range(nb):
            base = s * PB
            pm = psum.tile([128, B], fp32, tag="pm", name="pm")
            for t in range(KT):
                nc.tensor.matmul(
                    pm[0:FS, :],
                    inp[:, base + XW + 2 * t * FS : base + XW + 2 * (t + 1) * FS
                        ].bitcast(bf16),
                    inp[:, base + t * B : base + (t + 1) * B].bitcast(x_dt),
                    start=(t == 0),
                    stop=(t == KT - 1),
                )
            if mode != "nocopy":
                nc.vector.tensor_scalar(
                    out_sb[0:FS, u * B : (u + 1) * B], pm[0:FS, :], 0.0, None,
                    mybir.AluOpType.add,
                )
            u += 1
        if mode in ("nocopy", "noout"):
            continue
        out_eng = nc.gpsimd if g < len(group_sizes) - 1 else nc.sync
        out_eng.dma_start(
            out_d.ap()[:, out_lo * B : u * B],
            out_sb[0:FS, out_lo * B : u * B],
        )
        out_lo = u


def _declare_io(nc, mybir, unroll=UNROLL):
    out_dt = mybir.dt.float32 if OUT_F32 else mybir.dt.bfloat16
    # one packed input record: x^T fp8 bytes then W' bf16 bytes
    inp_d = nc.dram_tensor("inp", [128, PB], mybir.dt.int8,
                           kind="ExternalInput")
    # out[f_local, u*B + b] — final output slice (pre-bias) of execution u
    out_d = nc.dram_tensor("out", [FS, max(unroll, 1) * B], out_dt,
                           kind="ExternalOutput")
    return inp_d, out_d


G_P = 8          # bodies per pipeline tick (one input DMA per tick)
U_P = 4          # ticks per steady-state block (all-engine barrier cadence)


def _build_pipelined(nc, tc, bass, mybir, inp_d, out_d, n_iter, mode,
                     group=None, unroll=None):
    import contextlib

    group = G_P if group is None else group
    unroll = U_P if unroll is None else unroll

    fp32 = mybir.dt.float32
    bf16 = mybir.dt.bfloat16
    i8 = mybir.dt.int8
    x_dt = mybir.dt.float8e3
    out_dt = fp32 if OUT_F32 else bf16
    assert n_iter % group == 0, (n_iter, group)
    n_ticks = n_iter // group

    with (
        tc.tile_pool(name="pipe", bufs=1) as pool,
        tc.tile_pool(name="psum", bufs=2, space=bass.MemorySpace.PSUM) as psum,
    ):
        ctr = {"load": 0}

        def load(pipe, iv):
            k = ctr["load"]
            ctr["load"] += 1
            t = pipe.intermediate_tile([128, group * PB], i8, name="inp")
            ring = nc.sync if k % 2 == 0 else nc.scalar
            # each body streams its own copy of the input record from HBM —
            # the source AP repeats the DRAM region (stride-0 outer dim)
            src = inp_d.ap().unsqueeze(1).broadcast_to([128, group, PB])
            ring.dma_start(t[:].rearrange("p (g c) -> p g c", g=group), src)
            return t

        def compute(pipe, iv, t):
            o = pipe.intermediate_tile([128, group * B], out_dt, name="osb")
            nkt = KT // 2 if mode == "halfmm" else KT
            for s in range(group):
                base = s * PB
                pm = psum.tile([128, B], fp32, tag="pm", name="pm")
                for k in range(nkt):
                    nc.tensor.matmul(
                        pm[0:FS, :],
                        t[:, base + XW + 2 * k * FS : base + XW + 2 * (k + 1) * FS
                          ].bitcast(bf16),
                        t[:, base + k * B : base + (k + 1) * B].bitcast(x_dt),
                        start=(k == 0),
                        stop=(k == nkt - 1),
                    )
                nc.vector.tensor_scalar(
                    o[0:FS, s * B : (s + 1) * B], pm[0:FS, :], 0.0, None,
                    mybir.AluOpType.add,
                )
            return o

        def store(pipe, iv, o):
            if mode == "hwout":
                k = ctr["store"]
                ctr["store"] += 1
                eng = nc.scalar if k % 2 == 0 else nc.sync
            else:
                eng = nc.gpsimd
            eng.dma_start(out_d.ap()[:, 0 : group * B], o[0:FS, :])

        ctr["store"] = 0
        stages = {
            "dma": [load],
            "nostore": [load, compute],
            "halfmm": [load, compute],
            "full": [load, compute, store],
            "hwout": [load, compute, store],
        }[mode]
        tc.For_i_pipelined(stages, 0, n_ticks, pool=pool, unroll=unroll)


def _build_program(n_iter=1, mode="full", unroll=UNROLL):
    import concourse.bass as bass
    import concourse.tile as tile
    from concourse import bacc, mybir

    nc = bacc.Bacc("TRN2", target_bir_lowering=False, debug=False)
    PIPE_MODES = ("full", "dma", "nostore", "halfmm", "hwout")
    if n_iter == 1:
        io = _declare_io(nc, mybir, unroll=1)
    elif mode in PIPE_MODES:
        io = _declare_io(nc, mybir, unroll=G_P)
    else:
        io = _declare_io(nc, mybir, unroll=unroll)

    with tile.TileContext(nc) as tc:
        if n_iter > 1 and mode in PIPE_MODES:
            _build_pipelined(nc, tc, bass, mybir, *io, n_iter, mode)
        else:
            bmode = mode[1:] if mode.startswith("b") else mode
            with (
                tc.tile_pool(name="const", bufs=1) as const,
                tc.tile_pool(name="opool", bufs=1) as opool,
                tc.tile_pool(name="psum", bufs=2,
                             space=bass.MemorySpace.PSUM) as psum,
            ):
                pools = (const, opool, psum)
                if n_iter == 1:
                    _emit_iter(nc, tc, bass, mybir, pools, *io, mode=bmode,
                               unroll=1)
                else:
                    assert n_iter % unroll == 0, (n_iter, unroll)
                    with tc.For_i(0, n_iter // unroll, 1):
                        _emit_iter(nc, tc, bass, mybir, pools, *io, mode=bmode,
                                   unroll=unroll)

    nc.compile()
    return nc


def _prep_inputs(x, binary, scale):
    xf = np.asarray(x, dtype=np.float32).reshape(B, NX)
    # xT[p, t*B + b] = xf[b, t*128 + p]
    xT = np.ascontiguousarray(
        xf.T.reshape(KT, 128, B).transpose(1, 0, 2)
    ).reshape(128, XW).astype(ml_dtypes.float8_e3m4)

    bins = np.asarray(binary)[0].astype(np.uint8)        # [8, 96, 768]
    bits = np.unpackbits(bins[:, :, :, None], axis=3)    # [..., p] = bit (7-p)
    sgn = bits.astype(np.float32) * 2.0 - 1.0            # [8k, 96m, 768f, 8p]
    sc = np.asarray(scale, dtype=np.float32)[0]          # [8, 768]
    W = np.einsum("kmfp,kf->mpf", sgn, sc).reshape(NX, NF)
    Wr = W.reshape(KT, 128, NCORES, FS)                  # [t, p, j, f]

    in_maps = []
    for j in range(NCORES):
        wj = np.ascontiguousarray(Wr[:, :, j, :].transpose(1, 0, 2)).reshape(
            128, WC
        ).astype(ml_dtypes.bfloat16)
        packed = np.concatenate(
            [xT.view(np.int8), wj.view(np.int8)], axis=1
        )  # [128, PB]
        in_maps.append({"inp": np.ascontiguousarray(packed)})
    return in_maps


def kernel(x, scale, binary, bias, _trace=False):
    from concourse.bass_utils import run_bass_kernel_spmd

    if "nc" not in _CACHE:
        _CACHE["nc"] = _build_program()
    nc = _CACHE["nc"]

    in_maps = _prep_inputs(x, binary, scale)
    res = run_bass_kernel_spmd(nc, in_maps, core_ids=list(range(NCORES)), trace=_trace)
    _CACHE["last_result"] = res

    outT = np.concatenate(
        [np.asarray(res.results[j]["out"])[:, 0:B].astype(np.float32)
         for j in range(NCORES)],
        axis=0,
    )  # [768, 256]
    out = outT.T + np.asarray(bias, dtype=np.float32)[None, :]
    return out.reshape(4, 64, NF).astype(np.float32)
